# revision 24
# baseline (speedup 1.0000x reference)
"""Multi-head self-attention (B=2, S=2048, E=1024, H=16) on 8 Trainium2 cores.

Sharding: tensor-parallel over heads -- 2 heads per core. Each core:
  - computes Q/K/V projections for its 128 E-dims (d-major layouts),
  - runs attention for its (2 heads x 2 batches),
  - emits a partial output projection (contraction over its 128 dims of Wo).
Host sums the 8 partials and adds the output bias.

All matmuls run in "transposed" space so the big P = softmax(QK^T) matrix
never needs an on-chip transpose:
  ST[k,q] = K @ Q^T        (lhsT = K^T tile, rhs = Q^T tile)
  PT      = exp(ST)        (ScalarE, straight from PSUM)
  attn^T  = V'^T P^T       (lhsT = k-major V chunk with a ones column ->
                            row 64 of the psum is the softmax row-sum)
Scale 1/sqrt(dh)=1/8 is folded into Wq/bq on the host; the V bias is applied
inside the V projection (valid because softmax rows sum to 1).
"""

import sys

sys.path.insert(0, "/opt/trn_rl_repo")

import numpy as np

B = 2
S = 2048
E = 1024
H = 16
DH = 64
NCORES = 8
HPC = H // NCORES  # heads per core = 2
LOC = HPC * DH     # local E dims per core = 128

_CACHED = {}


def _split_waits(nc):
    """Walrus in this toolchain accepts at most ONE sync wait per instruction.
    Split any multi-wait instruction into single-wait NoOps on the same engine
    placed immediately before it (sequencer stalls are order-equivalent)."""
    import concourse.mybir as mybir

    nid = 0
    for blk in nc.m.functions[0].blocks:
        out = []
        changed = False
        for inst in blk.instructions:
            si = inst.sync_info
            if si is not None and len(si.on_wait) > 1:
                waits = list(si.on_wait)
                for w in waits[:-1]:
                    nid += 1
                    n = mybir.InstNoOp(name=f"I-waitsplit-{nid}", ins=[], outs=[])
                    n.engine = inst.engine
                    n.sync_info = mybir.SyncInfo(on_wait=[w], on_update=[])
                    out.append(n)
                inst.sync_info = mybir.SyncInfo(
                    on_wait=[waits[-1]], on_update=list(si.on_update)
                )
                changed = True
            out.append(inst)
        if changed:
            blk.instructions = out
    return nc


def build_nc(s=S, debug=False):
    """Build the per-core Bass program. `s` = sequence length (parametric so
    CoreSim checks can run on a smaller config)."""
    import concourse.bass as bass
    import concourse.mybir as mybir
    import concourse.tile as tile
    from concourse.masks import make_identity

    F32 = mybir.dt.float32
    F32R = mybir.dt.float32r
    BF16 = mybir.dt.bfloat16
    r = B * s              # total rows
    NCH = r // 512         # 512-wide column chunks over rows
    KT = s // 128          # 128-key tiles per batch
    QC = s // 512          # 512-wide q chunks per batch
    NTR = r // 128         # 128-row transpose tiles

    nc = bass.Bass()

    if debug:
        dbg = {
            "dbg_qt": nc.declare_dram_parameter("dbg_qt", [128, r], BF16, isOutput=True),
            "dbg_kt": nc.declare_dram_parameter("dbg_kt", [128, r], BF16, isOutput=True),
            "dbg_vp0": nc.declare_dram_parameter("dbg_vp0", [128, NTR, 65], BF16, isOutput=True),
            "dbg_attn": nc.declare_dram_parameter("dbg_attn", [128, r], F32R, isOutput=True),
        }

    xT = nc.declare_dram_parameter("xT", [r // 512, 128, 8, 512], F32R, isOutput=False)
    wq = nc.declare_dram_parameter("wq", [128, 8, 128], F32R, isOutput=False)
    wk = nc.declare_dram_parameter("wk", [128, 8, 128], F32R, isOutput=False)
    wv = nc.declare_dram_parameter("wv", [128, 8, 128], F32R, isOutput=False)
    bq = nc.declare_dram_parameter("bq", [128, 1], F32, isOutput=False)
    bk = nc.declare_dram_parameter("bk", [128, 1], F32, isOutput=False)
    bv = nc.declare_dram_parameter("bv", [128, 1], F32, isOutput=False)
    wo = nc.declare_dram_parameter("wo", [128, E], F32R, isOutput=False)
    selc = nc.declare_dram_parameter("selc", [128, 128], F32R, isOutput=False)
    outp = nc.declare_dram_parameter("out", [r, E], F32, isOutput=True)

    with tile.TileContext(nc) as tc:
        with (
            tc.tile_pool(name="consts", bufs=1) as consts,
            tc.tile_pool(name="xt", bufs=3) as xt_pool,
            tc.tile_pool(name="qkv", bufs=1) as qkv_pool,
            tc.tile_pool(name="vtmp", bufs=2) as vtmp_pool,
            tc.tile_pool(name="pt", bufs=6) as pt_pool,
            tc.tile_pool(name="small", bufs=2) as small_pool,
            tc.tile_pool(name="bcs", bufs=2) as bcs_pool,
            tc.tile_pool(name="osb", bufs=3) as osb_pool,
            tc.tile_pool(name="ps_mm", bufs=2, space="PSUM") as ps_mm,
            tc.tile_pool(name="ps_st", bufs=2, space="PSUM") as ps_st,
            tc.tile_pool(name="ps_pv", bufs=2, space="PSUM") as ps_pv,
        ):
            # first x chunk is on the critical path to the first matmul:
            # DMA it (in two queue-parallel halves) before the constants
            xt0 = xt_pool.tile([128, 8, 512], F32R, tag="xt", name="xt0")
            nc.sync.dma_start(xt0[:, 0:4, :], xT[0, :, 0:4, :])
            nc.sync.dma_start(xt0[:, 4:8, :], xT[0, :, 4:8, :])

            # ---- constants ----
            wq_sb = consts.tile([128, 8, 128], F32R, tag="wq")
            wk_sb = consts.tile([128, 8, 128], F32R, tag="wk")
            wv_sb = consts.tile([128, 8, 128], F32R, tag="wv")
            bq_sb = consts.tile([128, 1], F32, tag="bq")
            bk_sb = consts.tile([128, 1], F32, tag="bk")
            bv_sb = consts.tile([128, 1], F32, tag="bv")
            wo_sb = consts.tile([128, E], F32R, tag="wo")
            selc_sb = consts.tile([128, 128], F32R, tag="selc")
            ident = consts.tile([128, 128], F32, tag="ident")
            nc.sync.dma_start(wq_sb[:], wq[:])
            nc.sync.dma_start(wk_sb[:], wk[:])
            nc.sync.dma_start(wv_sb[:], wv[:])
            nc.sync.dma_start(bq_sb[:], bq[:])
            nc.sync.dma_start(bk_sb[:], bk[:])
            nc.sync.dma_start(bv_sb[:], bv[:])
            nc.sync.dma_start(wo_sb[:], wo[:])
            nc.sync.dma_start(selc_sb[:], selc[:])
            make_identity(nc, ident[:])

            # persistent activations
            qt_sb = qkv_pool.tile([128, r], BF16, tag="qt")     # Q^T  (scaled)
            kt_sb = qkv_pool.tile([128, r], BF16, tag="kt")     # K^T
            vp0 = qkv_pool.tile([128, NTR, 65], BF16, tag="vp0")  # k-major V' head 0
            vp1 = qkv_pool.tile([128, NTR, 65], BF16, tag="vp1")  # k-major V' head 1
            attn_sb = qkv_pool.tile([128, r], F32R, tag="attn")   # normalized attn^T
            nc.vector.memset(vp0[:, :, 64], 1.0)
            nc.vector.memset(vp1[:, :, 64], 1.0)

            # ~5us of dummy matmuls at start: runs while the first input DMA
            # is in flight and lifts the PE HAM clock-gate to 8/8 (2.4 GHz)
            # before the real matmuls begin.
            warm_sb = consts.tile([128, 512], BF16, tag="warm")
            nc.vector.memset(warm_sb[:], 0.0)
            warm_ps = ps_mm.tile([128, 512], F32, tag="mm", name="warmps")
            for wi in range(24):
                nc.tensor.matmul(
                    warm_ps[:],
                    warm_sb[:, 0:128],
                    warm_sb[:],
                    start=(wi == 0),
                    stop=(wi == 23),
                )

            # ---- phase A: projections (d-major) + V transpose to k-major ----
            # V transposes are deferred by one chunk so the PE never stalls
            # on the freshly-written vtmp (its DVE bias-copy is one proj-group
            # old by the time the transposes dispatch).
            def emit_transposes(nch_v, vtmp_v):
                for t4 in range(4):
                    trp = ps_st.tile([128, 128], F32, tag="st")
                    nc.tensor.transpose(
                        trp[:], vtmp_v[:, t4 * 128 : (t4 + 1) * 128], ident[:]
                    )
                    tg = nch_v * 4 + t4
                    nc.vector.tensor_copy(vp0[:, tg, 0:64], trp[:, 0:64])
                    nc.vector.tensor_copy(vp1[:, tg, 0:64], trp[:, 64:128])

            pending_tr = None
            for nch in range(NCH):
                if nch == 0:
                    xt = xt0
                else:
                    xt = xt_pool.tile([128, 8, 512], F32R, tag="xt")
                    nc.sync.dma_start(xt[:, 0:4, :], xT[nch, :, 0:4, :])
                    nc.sync.dma_start(xt[:, 4:8, :], xT[nch, :, 4:8, :])
                c0 = nch * 512
                for w_sb, b_sb, dest in (
                    (wq_sb, bq_sb, qt_sb),
                    (wk_sb, bk_sb, kt_sb),
                    (wv_sb, bv_sb, None),
                ):
                    ps = ps_mm.tile([128, 512], F32, tag="mm")
                    for kc in range(8):
                        nc.tensor.matmul(
                            ps[:],
                            w_sb[:, kc, :],
                            xt[:, kc, :],
                            start=(kc == 0),
                            stop=(kc == 7),
                        )
                    if dest is not None:
                        nc.vector.tensor_scalar_add(
                            dest[:, c0 : c0 + 512], ps[:], b_sb[:, 0:1]
                        )
                    else:
                        vtmp = vtmp_pool.tile([128, 512], F32, tag="vtmp")
                        nc.vector.tensor_scalar_add(vtmp[:], ps[:], b_sb[:, 0:1])
                        if pending_tr is not None:
                            emit_transposes(*pending_tr)
                        pending_tr = (nch, vtmp)
            emit_transposes(*pending_tr)

            # ---- phase B: attention + phase C: partial output projection ----
            for b in range(B):
                for qc in range(QC):
                    gq = b * s + qc * 512
                    # two PV accumulators (one per head), filled in one
                    # interleaved kt loop so the two K=64 score matmuls sit
                    # adjacent in the PE stream (row-group concurrency) and
                    # one wide exp covers both heads.
                    pvp0 = ps_pv.tile([65, 512], F32, tag="pv", name="pvp0")
                    pvp1 = ps_pv.tile([65, 512], F32, tag="pv", name="pvp1")
                    pv_tiles = [pvp0, pvp1]
                    # software-pipelined one kt deep: PV(kt-1) is emitted after
                    # ST(kt), so its wait on exp(kt-1) is already satisfied.
                    def emit_pv(kt_v, pt_v):
                        for h in range(2):
                            nc.tensor.matmul(
                                pv_tiles[h][:],
                                (vp0 if h == 0 else vp1)[:, b * KT + kt_v, :],
                                pt_v[:, h * 512 : h * 512 + 512],
                                start=(kt_v == 0),
                                stop=(kt_v == KT - 1),
                            )

                    pending_pv = []
                    for kt in range(KT):
                        kcol = b * s + kt * 128
                        stp = ps_st.tile([128, 1024], F32, tag="st")
                        for h in range(2):
                            p0 = h * 64
                            nc.tensor.matmul(
                                stp[:, h * 512 : h * 512 + 512],
                                kt_sb[p0 : p0 + 64, kcol : kcol + 128],
                                qt_sb[p0 : p0 + 64, gq : gq + 512],
                                start=True,
                                stop=True,
                            )
                        pt = pt_pool.tile([128, 1024], BF16, tag="pt")
                        nc.scalar.activation(
                            pt[:], stp[:], mybir.ActivationFunctionType.Exp
                        )
                        pending_pv.append((kt, pt))
                        if len(pending_pv) > 2:
                            emit_pv(*pending_pv.pop(0))
                    for args in pending_pv:
                        emit_pv(*args)
                    # normalize: rowsums live at psum row 64 of each pv tile.
                    # Cast-copy sums to SBUF, copy the unnormalized attn bands
                    # to SBUF right away (releases the pv psum tiles early so
                    # the next unit's PV accumulation can start), then
                    # matmul-broadcast both sums into one [128,512] psum bank,
                    # reciprocal it once, and scale the bands in place.
                    rbase = (64, 32)  # selector rows: h0 sums via row 64, h1 via 32
                    rshs = []
                    for h in range(2):
                        rb = rbase[h]
                        rsh = small_pool.tile([65, 512], F32R, tag="rs")
                        nc.vector.tensor_copy(rsh[rb : rb + 1, :], pv_tiles[h][64:65, :])
                        rshs.append(rsh)
                    for h in range(2):
                        p0 = h * 64
                        nc.vector.tensor_copy(
                            attn_sb[p0 : p0 + 64, gq : gq + 512], pv_tiles[h][0:64, :]
                        )
                    bcp = ps_st.tile([128, 1024], F32, tag="st", name="bcp")
                    for h in range(2):
                        rb = rbase[h]
                        nc.tensor.matmul(
                            bcp[:, 0:512],
                            selc_sb[rb : rb + 1, :],
                            rshs[h][rb : rb + 1, :],
                            start=(h == 0),
                            stop=(h == 1),
                        )
                    bcs = bcs_pool.tile([128, 512], F32, tag="bcs")
                    nc.vector.reciprocal(bcs[:], bcp[:, 0:512])
                    for h in range(2):
                        p0 = h * 64
                        nc.vector.tensor_tensor(
                            attn_sb[p0 : p0 + 64, gq : gq + 512],
                            attn_sb[p0 : p0 + 64, gq : gq + 512],
                            bcs[p0 : p0 + 64, :],
                            mybir.AluOpType.mult,
                        )
                    # partial out projection for these 512 rows
                    for qb in range(4):
                        col = gq + qb * 128
                        for no2 in range(2):
                            ops = ps_mm.tile([128, 512], F32, tag="mm")
                            nc.tensor.matmul(
                                ops[:],
                                attn_sb[:, col : col + 128],
                                wo_sb[:, no2 * 512 : (no2 + 1) * 512],
                                start=True,
                                stop=True,
                            )
                            osb = osb_pool.tile([128, 512], F32, tag="osb")
                            nc.vector.tensor_copy(osb[:], ops[:])
                            nc.sync.dma_start(
                                outp[col : col + 128, no2 * 512 : (no2 + 1) * 512],
                                osb[:],
                            )
            if debug:
                nc.sync.dma_start(dbg["dbg_qt"][:], qt_sb[:])
                nc.sync.dma_start(dbg["dbg_kt"][:], kt_sb[:])
                nc.sync.dma_start(dbg["dbg_vp0"][:], vp0[:])
                nc.sync.dma_start(dbg["dbg_attn"][:], attn_sb[:])
    return nc


def _prep_inputs(inputs, Wq, bq, Wk, bk, Wv, bv, Wo, bo, s=S):
    """Host-side shard + relayout. Returns (in_maps, bo)."""
    r = B * s
    x = np.ascontiguousarray(inputs, dtype=np.float32).reshape(r, E)
    # [E, r] -> per-512-chunk contiguous tiles [NCH, 128(part), 8(kc), 512]
    xT = np.ascontiguousarray(
        x.T.reshape(8, 128, r // 512, 512).transpose(2, 1, 0, 3)
    )

    selc = np.zeros((128, 128), dtype=np.float32)
    selc[64, 0:64] = 1.0
    selc[32, 64:128] = 1.0

    def wslice(W, c):
        # W[c*128:(c+1)*128, :] transposed -> [E, 128] -> [128(part), 8, 128]
        wt = np.ascontiguousarray(
            W[c * 128 : (c + 1) * 128, :].T.reshape(8, 128, 128).transpose(1, 0, 2)
        )
        return wt

    in_maps = []
    for c in range(NCORES):
        m = {
            "xT": xT,
            "wq": wslice(Wq, c) * 0.125,
            "wk": wslice(Wk, c),
            "wv": wslice(Wv, c),
            "bq": (bq[c * 128 : (c + 1) * 128] * 0.125).reshape(128, 1).astype(np.float32),
            "bk": bk[c * 128 : (c + 1) * 128].reshape(128, 1).astype(np.float32),
            "bv": bv[c * 128 : (c + 1) * 128].reshape(128, 1).astype(np.float32),
            "wo": np.ascontiguousarray(Wo[:, c * 128 : (c + 1) * 128].T),
            "selc": selc,
        }
        in_maps.append(m)
    return in_maps


def _get_nc(s=S):
    if s not in _CACHED:
        _CACHED[s] = _split_waits(build_nc(s))
    return _CACHED[s]


def kernel(
    inputs, Wq, bq, Wk, bk, Wv, bv, Wo, bo, _trace=False, _result_box=None
):
    from concourse.bass_utils import run_bass_kernel_spmd

    nc = _get_nc(S)
    in_maps = _prep_inputs(inputs, Wq, bq, Wk, bk, Wv, bv, Wo, bo)
    res = run_bass_kernel_spmd(nc, in_maps, list(range(NCORES)), trace=_trace)
    if _result_box is not None:
        _result_box.append(res)
    acc = np.zeros((B * S, E), dtype=np.float32)
    for rmap in res.results:
        acc += rmap["out"]
    acc += bo.astype(np.float32)[None, :]
    return acc.reshape(B, S, E)
